# revision 26
# baseline (speedup 1.0000x reference)
"""Trainium2 Bass kernel for nn_ALIGNN (ResGatedGraphConv GNN, 3 layers).

Strategy (8 NeuronCores, SPMD):
  - Nodes are globally sorted by in-degree and dealt round-robin to cores so
    every core holds 12500 nodes whose degree profile is identical across
    cores -> one uniform program for all cores (SPMD requirement).
  - Per core, nodes form 64-wide tiles; each tile gets a "slot grid" sized by
    the tile's max degree (uniform across cores by construction).  Edges are
    laid out slot-innermost (edge position = n*w + s inside a slot-group of
    width w), so the per-node message sum is an innermost-axis tensor_reduce.
  - Per-edge compute is FEATURE-MAJOR ([feature, edge] tiles):
      PSUM X[0:64 ,:] = 2*e + k'[dst] + q'[src]     (gate argument)
      PSUM X[64:128,:] = e + v'[src]                 (value argument)
    built by three accumulating matmuls:
      MM1: lhsT=[2We|We] (50x128)      rhs = edge_attr^T chunk (moving)
      MM2: lhsT=k' node-major slice    rhs = block-diag expansion constant
      MM3: lhsT=gathered qv rows       rhs = I128   (transpose + add in PE)
    then gate = ACT silu(X[0:64]), msg = DVE gate * X[64:128],
    agg = DVE tensor_reduce(msg, innermost slot axis).
  - q'/v' for all nodes live in a bf16 [8*12544, 128] table ([q'|v'] rows,
    biases folded in), rebuilt each layer from local h via matmul and
    AllGather; per-edge rows are fetched with an int32 indirect DMA gather.
  - BatchNorm: local sum/sumsq -> [64,2] AllReduce -> scale/shift applied by
    one ACT op.  Mean-pool: PE transpose of h + matmul with 0/1 graph matrix,
    [64,512] AllReduce, then the 2-layer head (all cores redundantly).
"""

import os
import sys

import numpy as np

sys.path.insert(0, "/opt/trn_rl_repo")

import ml_dtypes

import concourse.bass as bass
import concourse.mybir as mybir
import concourse.tile as tile
from concourse import bacc
from concourse.bass import IndirectOffsetOnAxis
from concourse.bass_utils import run_bass_kernel_spmd

BF16 = mybir.dt.bfloat16
F32 = mybir.dt.float32
I32 = mybir.dt.int32
I16 = mybir.dt.int16
AF = mybir.ActivationFunctionType
ALU = mybir.AluOpType

P_CORES = 8
BN_EPS = 1e-5


# ----------------------------------------------------------------------------
# host-side planning
# ----------------------------------------------------------------------------
class Plan:
    pass


def _build_plan(edge_index, batch_idx, n_nodes, n_graphs):
    """Pure-numpy partitioning/geometry plan. Uniform across cores."""
    pl = Plan()
    src = np.asarray(edge_index[0], dtype=np.int64)
    dst = np.asarray(edge_index[1], dtype=np.int64)
    batch_idx = np.asarray(batch_idx, dtype=np.int64)

    deg = np.bincount(dst, minlength=n_nodes)
    order = np.argsort(deg, kind="stable")  # ascending degree
    rank_of = np.empty(n_nodes, dtype=np.int64)
    rank_of[order] = np.arange(n_nodes)

    core_of = rank_of % P_CORES
    local_of = rank_of // P_CORES  # 0..12499
    n_loc = n_nodes // P_CORES  # 12500
    n_tiles = (n_loc + 63) // 64  # 196
    npad = n_tiles * 64  # 12544
    table_row = core_of * npad + local_of

    # uniform tile degrees from the global sorted sequence
    d_t = np.zeros(n_tiles, dtype=np.int64)
    for t in range(n_tiles):
        win = order[t * 64 * P_CORES: (t + 1) * 64 * P_CORES]
        d = int(deg[win].max()) if len(win) else 2
        d = max(d, 2)
        d_t[t] = d + (d % 2)  # round up to even

    # slot-groups per tile: widths w (even, <=16)
    groups = []  # (tile, w, slot_base, edge_base)
    ebase = 0
    tile_groups = [[] for _ in range(n_tiles)]
    for t in range(n_tiles):
        rem, sbase = int(d_t[t]), 0
        while rem > 0:
            w = min(rem, 16)
            gi = len(groups)
            groups.append((t, w, sbase, ebase))
            tile_groups[t].append(gi)
            ebase += 64 * w
            sbase += w
            rem -= w
    e_pad = ebase

    # supergroups: consecutive groups packed to <=4096 edges
    sgs = []  # (edge_base, n_edges, [group indices])
    cur, cur_base, cur_len = [], 0, 0
    for gi, (t, w, sb, eb) in enumerate(groups):
        ne = 64 * w
        if cur and cur_len + ne > 4096:
            sgs.append((cur_base, cur_len, cur))
            cur, cur_base, cur_len = [], eb, 0
        if not cur:
            cur_base = eb
        cur.append(gi)
        cur_len += ne
    if cur:
        sgs.append((cur_base, cur_len, cur))

    pl.n_nodes, pl.n_graphs = n_nodes, n_graphs
    pl.n_loc, pl.n_tiles, pl.npad, pl.e_pad = n_loc, n_tiles, npad, e_pad
    pl.d_t, pl.groups, pl.tile_groups, pl.sgs = d_t, groups, tile_groups, sgs
    pl.zero_row = n_loc  # core-0 pad row: always-zero table row
    pl.order, pl.core_of, pl.local_of, pl.table_row = order, core_of, local_of, table_row
    pl.src, pl.dst, pl.batch = src, dst, batch_idx
    pl.ws = sorted(set(w for (_, w, _, _) in groups))

    # per-core edge placement (needed for uniform gather-bank capacities)
    pl.placed = [_place_edges_for_core(pl, c) for c in range(P_CORES)]
    BANK = 32768
    table_rows = P_CORES * npad
    pl.n_banks = (table_rows + BANK - 1) // BANK
    # an always-zero table row inside each bank (some core's pad region)
    pl.bank_zero = []
    for b in range(pl.n_banks):
        zr = None
        for c in range(P_CORES):
            for r in range(c * npad + n_loc, (c + 1) * npad):
                if b * BANK <= r < (b + 1) * BANK:
                    zr = r - b * BANK
                    break
            if zr is not None:
                break
        assert zr is not None, f"no zero row in bank {b}"
        pl.bank_zero.append(zr)
    # per (sg, bank): uniform dest capacity (128-blocks) = max over cores
    pl.sg_bank = []  # list per sg: (caps[b] in blocks, offsets[b] in blocks)
    for (eb, ne, gis) in sgs:
        caps = []
        for b in range(pl.n_banks):
            mx = 0
            for c in range(P_CORES):
                srow = pl.placed[c][1][eb:eb + ne]
                mx = max(mx, int((srow // BANK == b).sum()))
            caps.append((mx + 127) // 128)
        offs = np.concatenate([[0], np.cumsum(caps)])[:-1].tolist()
        pl.sg_bank.append((caps, offs))
    return pl


def _place_edges_for_core(pl, c):
    """Return (ea_eid[e_pad] int64 (-1 dummy), src_row[e_pad] int32)."""
    sel = np.nonzero(pl.core_of[pl.dst] == c)[0]
    li = pl.local_of[pl.dst[sel]]
    o = np.argsort(li, kind="stable")
    sel, li = sel[o], li[o]
    # position within each node's edge list
    first = np.searchsorted(li, np.arange(pl.n_loc))
    pos = np.arange(len(li)) - first[li]

    t = li // 64
    n = li % 64
    ea_eid = np.full(pl.e_pad, -1, dtype=np.int64)
    src_row = np.full(pl.e_pad, pl.zero_row, dtype=np.int32)

    # map slot j -> (group, s) per tile via per-tile group tables
    flat = np.empty(len(li), dtype=np.int64)
    for ti in range(pl.n_tiles):
        m = np.nonzero(t == ti)[0]
        if len(m) == 0:
            continue
        jj = pos[m]
        gidx = np.zeros(len(m), dtype=np.int64)
        gs = np.zeros(len(m), dtype=np.int64)
        for gi in pl.tile_groups[ti]:
            (_, w, sb, eb) = pl.groups[gi]
            inw = (jj >= sb) & (jj < sb + w)
            gidx[inw] = gi
            gs[inw] = jj[inw] - sb
        ebs = np.array([pl.groups[g][3] for g in gidx])
        wg = np.array([pl.groups[g][1] for g in gidx])
        flat[m] = ebs + n[m] * wg + gs
    ea_eid[flat] = sel
    src_row[flat] = pl.table_row[pl.src[sel]].astype(np.int32)
    return ea_eid, src_row


def _bf16(a):
    return np.asarray(a, dtype=np.float32).astype(ml_dtypes.bfloat16)


def _wrap16(vals):
    """int16 list (len % 16 == 0) -> [128, n/16] wrapped+replicated layout."""
    n = len(vals)
    assert n % 16 == 0
    w = np.asarray(vals, np.int16).reshape(n // 16, 16).T
    return np.ascontiguousarray(np.tile(w, (8, 1)))


def _host_inputs(pl, x, edge_attr, weights):
    """Build per-core input dicts (shared entries replicated)."""
    (pre_w, pre_b, Wk, bk, Wq, bq, Wv, bv, We, Wskip, conv_bias,
     bn_gamma, bn_beta, post_w, post_b, out_w, out_b) = weights
    L = Wk.shape[0]
    npad, n_loc = pl.npad, pl.n_loc

    shared = {}
    # [2We | We] per layer, stacked on columns: [50, L*128]
    WeC = np.concatenate(
        [np.concatenate([2.0 * We[l], We[l]], axis=1) for l in range(L)], axis=1)
    shared["WeC"] = _bf16(WeC)
    shared["Wk_all"] = _bf16(np.concatenate([Wk[l] for l in range(L)], axis=1))
    # Wqv aug: rows 0..63 = [Wq|Wv], row 64 = [bq+bk | bv]
    Wqv = np.concatenate(
        [np.concatenate(
            [np.concatenate([Wq[l], Wv[l]], axis=1),
             np.concatenate([bq[l] + bk[l], bv[l]])[None, :]], axis=0)
         for l in range(L)], axis=1)
    shared["Wqv_all"] = _bf16(Wqv)  # [65, L*128]
    Wsk = np.concatenate(
        [np.concatenate([Wskip[l], conv_bias[l][None, :]], axis=0)
         for l in range(L)], axis=1)
    shared["Wskip_all"] = _bf16(Wsk)  # [65, L*64]
    shared["preW"] = _bf16(np.concatenate([pre_w, pre_b[None, :]], axis=0))  # [93,64]
    shared["gamma"] = np.ascontiguousarray(bn_gamma.T).astype(np.float32)  # [64, L]
    shared["beta"] = np.ascontiguousarray(bn_beta.T).astype(np.float32)
    shared["post_w"] = _bf16(post_w)  # [64,64]
    shared["post_b"] = np.asarray(post_b, np.float32).reshape(64, 1)
    shared["out_w"] = _bf16(out_w)  # [64,1]

    cnts = np.bincount(pl.batch, minlength=pl.n_graphs).astype(np.float32)
    recip = 1.0 / np.maximum(cnts, 1.0)
    shared["recip"] = np.broadcast_to(recip[None, :], (64, pl.n_graphs)).astype(np.float32).copy()

    eaT_f = np.asarray(edge_attr, np.float32)
    xT_f = np.asarray(x, np.float32)

    per_core = []
    for c in range(P_CORES):
        d = dict(shared)
        ea_eid, src_row = pl.placed[c]
        eaT = np.zeros((50, pl.e_pad), dtype=ml_dtypes.bfloat16)
        m = ea_eid >= 0
        eaT[:, m] = _bf16(eaT_f[ea_eid[m]].T)
        d["eaT"] = eaT
        # two-stage gather indices
        BANK = 32768
        g1_cols, g2_cols = [], []
        for si, (eb, ne, gis) in enumerate(pl.sgs):
            caps, offs = pl.sg_bank[si]
            srow = src_row[eb:eb + ne]
            bank = srow // BANK
            pos_in_stage1 = np.zeros(ne, dtype=np.int16)
            for b in range(pl.n_banks):
                if caps[b] == 0:
                    continue
                cap = caps[b] * 128
                m = np.nonzero(bank == b)[0]
                v = np.full(cap, pl.bank_zero[b], dtype=np.int16)
                v[:len(m)] = (srow[m] - b * BANK).astype(np.int16)
                g1_cols.append(_wrap16(v))
                pos_in_stage1[m] = offs[b] * 128 + np.arange(len(m), dtype=np.int16)
            g2_cols.append(_wrap16(pos_in_stage1))
        d["g1idx"] = np.concatenate(g1_cols, axis=1)
        d["g2idx"] = np.concatenate(g2_cols, axis=1)
        # node features, transposed + ones row; pad cols zero
        xT = np.zeros((93, npad), dtype=ml_dtypes.bfloat16)
        glb = pl.order[np.arange(n_loc) * P_CORES + c]
        xT[:92, :n_loc] = _bf16(xT_f[glb].T)
        xT[92, :n_loc] = ml_dtypes.bfloat16(1.0)
        d["xT"] = xT
        # pool matrix [npad, n_graphs]
        G = np.zeros((npad, pl.n_graphs), dtype=ml_dtypes.bfloat16)
        G[np.arange(n_loc), pl.batch[glb]] = ml_dtypes.bfloat16(1.0)
        d["G"] = G
        per_core.append(d)
    return per_core


# ----------------------------------------------------------------------------
# device kernel builder
# ----------------------------------------------------------------------------
def _build_kernel(pl, out_b_val, debug_taps=False):
    L = 3
    npad, n_tiles, e_pad = pl.npad, pl.n_tiles, pl.e_pad
    n_pt = n_tiles // 2  # 128-node production tiles
    NG = pl.n_graphs
    max_cap_blocks = max(sum(caps) for (caps, offs) in pl.sg_bank)

    nc = bacc.Bacc(None, target_bir_lowering=False, num_devices=P_CORES)
    rg = [list(range(P_CORES))]

    # ---- parameters
    p_WeC = nc.declare_dram_parameter("WeC", [50, L * 128], BF16, isOutput=False)
    p_Wk = nc.declare_dram_parameter("Wk_all", [64, L * 64], BF16, isOutput=False)
    p_Wqv = nc.declare_dram_parameter("Wqv_all", [65, L * 128], BF16, isOutput=False)
    p_Wsk = nc.declare_dram_parameter("Wskip_all", [65, L * 64], BF16, isOutput=False)
    p_preW = nc.declare_dram_parameter("preW", [93, 64], BF16, isOutput=False)
    p_gamma = nc.declare_dram_parameter("gamma", [64, L], F32, isOutput=False)
    p_beta = nc.declare_dram_parameter("beta", [64, L], F32, isOutput=False)
    p_postw = nc.declare_dram_parameter("post_w", [64, 64], BF16, isOutput=False)
    p_postb = nc.declare_dram_parameter("post_b", [64, 1], F32, isOutput=False)
    p_outw = nc.declare_dram_parameter("out_w", [64, 1], BF16, isOutput=False)
    p_recip = nc.declare_dram_parameter("recip", [64, NG], F32, isOutput=False)
    p_eaT = nc.declare_dram_parameter("eaT", [50, e_pad], BF16, isOutput=False)
    g1_cols_total = sum(sum(caps) * 8 for (caps, offs) in pl.sg_bank)
    g2_cols_total = sum(ne // 16 for (_, ne, _) in pl.sgs)
    p_g1 = nc.declare_dram_parameter("g1idx", [128, g1_cols_total], I16, isOutput=False)
    p_g2 = nc.declare_dram_parameter("g2idx", [128, g2_cols_total], I16, isOutput=False)
    p_xT = nc.declare_dram_parameter("xT", [93, npad], BF16, isOutput=False)
    p_G = nc.declare_dram_parameter("G", [npad, NG], BF16, isOutput=False)
    p_out = nc.declare_dram_parameter("out", [1, NG], F32, isOutput=True)
    if debug_taps:
        d_qv0 = nc.declare_dram_parameter("dbg_qv0", [128, 4096], BF16, isOutput=True)
        d_agg0 = nc.declare_dram_parameter("dbg_agg0", [64, npad], F32, isOutput=True)
        d_h1 = nc.declare_dram_parameter("dbg_h1", [65, npad], BF16, isOutput=True)
        d_ksb0 = nc.declare_dram_parameter("dbg_ksb0", [64, n_tiles * 64], BF16, isOutput=True)
        d_tab0 = nc.declare_dram_parameter("dbg_tab0", [P_CORES * npad, 128], BF16, isOutput=True)

    # ---- internal DRAM
    qv_shard = nc.dram_tensor("qv_shard", [npad, 128], BF16)
    qv_table = nc.dram_tensor("qv_table", [P_CORES * npad, 128], BF16, addr_space="Shared")
    bn_in = nc.dram_tensor("bn_in", [64, 2], F32)
    bn_out = nc.dram_tensor("bn_out", [64, 2], F32, addr_space="Shared")
    pool_in = nc.dram_tensor("pool_in", [64, NG], F32)
    pool_out = nc.dram_tensor("pool_out", [64, NG], F32, addr_space="Shared")

    # inline constants
    ident128 = nc.inline_tensor(np.eye(128, dtype=ml_dtypes.bfloat16), name="I128")
    ident64 = nc.inline_tensor(np.eye(64, dtype=ml_dtypes.bfloat16), name="I64")
    r_pats = {}
    for w in pl.ws:
        R = np.zeros((64, 64 * w), dtype=ml_dtypes.bfloat16)
        for n in range(64):
            R[n, n * w:(n + 1) * w] = ml_dtypes.bfloat16(1.0)
        r_pats[w] = nc.inline_tensor(R, name=f"Rpat{w}")

    with tile.TileContext(nc) as tc:
        with (
            tc.tile_pool(name="persist", bufs=1) as pp,
            tc.tile_pool(name="stream", bufs=3) as sp,
            tc.tile_pool(name="gath", bufs=2) as gp,
            tc.tile_pool(name="small", bufs=2) as mp,
            tc.tile_pool(name="psA", bufs=2, space="PSUM") as psA,
            tc.tile_pool(name="psB", bufs=2, space="PSUM") as psB,
        ):
            # ---- persistent SBUF
            h = pp.tile([65, npad], BF16, tag="h")
            ksb = pp.tile([64, n_tiles * 64], BF16, tag="ksb")
            agg = pp.tile([64, npad], F32, tag="agg")
            WeC_sb = pp.tile([50, L * 128], BF16, tag="WeC")
            Wk_sb = pp.tile([64, L * 64], BF16, tag="Wk")
            Wqv_sb = pp.tile([65, L * 128], BF16, tag="Wqv")
            Wsk_sb = pp.tile([65, L * 64], BF16, tag="Wsk")
            preW_sb = pp.tile([93, 64], BF16, tag="preW")
            I128_sb = pp.tile([128, 128], BF16, tag="I128")
            I64_sb = pp.tile([64, 64], BF16, tag="I64")
            R_sb = {w: pp.tile([64, 64 * w], BF16, tag=f"R{w}", name=f"R{w}")
                    for w in pl.ws}
            gamma_sb = pp.tile([64, L], F32, tag="gamma")
            beta_sb = pp.tile([64, L], F32, tag="beta")

            nc.sync.dma_start(out=WeC_sb[:], in_=p_WeC[:, :])
            nc.sync.dma_start(out=Wk_sb[:], in_=p_Wk[:, :])
            nc.sync.dma_start(out=Wqv_sb[:], in_=p_Wqv[:, :])
            nc.sync.dma_start(out=Wsk_sb[:], in_=p_Wsk[:, :])
            nc.sync.dma_start(out=preW_sb[:], in_=p_preW[:, :])
            nc.sync.dma_start(out=I128_sb[:], in_=ident128[:, :])
            nc.sync.dma_start(out=I64_sb[:], in_=ident64[:, :])
            for w in pl.ws:
                nc.sync.dma_start(out=R_sb[w][:], in_=r_pats[w][:, :])
            nc.sync.dma_start(out=gamma_sb[:], in_=p_gamma[:, :])
            nc.sync.dma_start(out=beta_sb[:], in_=p_beta[:, :])

            # ---- pre-FC: h = silu(x @ pre_w + pre_b), xT streamed per chunk
            for c0 in range(0, npad, 512):
                cw = min(512, npad - c0)
                xt = sp.tile([93, 512], BF16, tag="xtile")
                nc.sync.dma_start(out=xt[:, :cw], in_=p_xT[:, c0:c0 + cw])
                ps = psB.tile([64, 512], F32, tag="nps")
                nc.tensor.matmul(ps[:, :cw], lhsT=preW_sb[:],
                                 rhs=xt[:, :cw], start=True, stop=True)
                nc.scalar.activation(h[0:64, c0:c0 + cw], ps[:, :cw], AF.Silu)
            nc.vector.memset(h[64:65, 0:pl.n_loc], 1.0)
            nc.vector.memset(h[64:65, pl.n_loc:npad], 0.0)
            nc.vector.memset(h[0:64, pl.n_loc:npad], 0.0)

            for l in range(L):
                # ---- node phase: qv table shard + k'
                for pt in range(n_pt):
                    cols = slice(pt * 128, pt * 128 + 128)
                    qps = psB.tile([128, 128], F32, tag="nps")
                    nc.tensor.matmul(qps[:], lhsT=h[:, cols],
                                     rhs=Wqv_sb[:, l * 128:(l + 1) * 128],
                                     start=True, stop=True)
                    qsb = mp.tile([128, 128], BF16, tag="qvsb")
                    nc.vector.tensor_copy(out=qsb[:], in_=qps[:])
                    nc.sync.dma_start(out=qv_shard[pt * 128:(pt + 1) * 128, :], in_=qsb[:])
                for t in range(n_tiles):
                    kps = psB.tile([64, 64], F32, tag="nps")
                    nc.tensor.matmul(kps[:], lhsT=h[0:64, t * 64:(t + 1) * 64],
                                     rhs=Wk_sb[:, l * 64:(l + 1) * 64],
                                     start=True, stop=True)
                    nc.vector.tensor_copy(out=ksb[:, t * 64:(t + 1) * 64], in_=kps[:])

                nc.gpsimd.collective_compute(
                    "AllGather", ALU.bypass, replica_groups=rg,
                    ins=[qv_shard.ap().opt()], outs=[qv_table.ap().opt()])
                if debug_taps and l == 0:
                    nc.sync.dma_start(out=d_tab0[:, :], in_=qv_table[:, :])
                    nc.sync.dma_start(out=d_ksb0[:, :], in_=ksb[:])

                # ---- edge phase
                k1off = 0
                k2off = 0
                for sgi, (eb, ne, gis) in enumerate(pl.sgs):
                    caps, offs = pl.sg_bank[sgi]
                    tot_blocks = sum(caps)
                    # stage 1: banked DRAM row-gathers (int16) -> edge-major
                    qv_t = gp.tile([128, max_cap_blocks, 128], BF16, tag="qvt")
                    for b in range(pl.n_banks):
                        if caps[b] == 0:
                            continue
                        nidx = caps[b] * 128
                        i1 = sp.tile([128, 512], I16, tag="idx1")
                        nc.sync.dma_start(out=i1[:, :nidx // 16],
                                          in_=p_g1[:, k1off:k1off + nidx // 16])
                        nc.gpsimd.dma_gather(
                            out_ap=qv_t[:, offs[b]:offs[b] + caps[b], :],
                            in_ap=qv_table[b * 32768:
                                           min((b + 1) * 32768, P_CORES * npad), :],
                            idxs_ap=i1[:, :nidx // 16],
                            num_idxs=nidx, num_idxs_reg=nidx, elem_size=128,
                            single_packet=False)
                        k1off += nidx // 16
                    # stage 2: SBUF re-gather into grid order, feature-major
                    i2 = sp.tile([128, 256], I16, tag="idx2")
                    nc.sync.dma_start(out=i2[:, :ne // 16],
                                      in_=p_g2[:, k2off:k2off + ne // 16])
                    qv_f = gp.tile([128, 4096], BF16, tag="qvf")
                    nc.gpsimd.dma_gather(
                        out_ap=qv_f[:, :ne].rearrange("p (o n) -> p o n", o=1),
                        in_ap=qv_t[:, :tot_blocks, :],
                        idxs_ap=i2[:, :ne // 16],
                        num_idxs=ne, num_idxs_reg=ne, elem_size=128,
                        transpose=True, sbuf_tokens_per_rank=128,
                        sbuf_free_dim_per_rank=256, single_packet=False)
                    k2off += ne // 16
                    if debug_taps and l == 0 and sgi == 0:
                        nc.sync.dma_start(out=d_qv0[:, :ne], in_=qv_f[:, :ne])
                    jg = 0  # 128-col block offset within supergroup
                    for gi in gis:
                        (t, w, sb, geb) = pl.groups[gi]
                        gne = 64 * w
                        X = psA.tile([128, 1024], F32, tag="X")
                        ea_t = sp.tile([50, 1024], BF16, tag="ea")
                        nc.sync.dma_start(out=ea_t[:, :gne], in_=p_eaT[:, geb:geb + gne])
                        for c0 in range(0, gne, 512):
                            cw = min(512, gne - c0)
                            nc.tensor.matmul(
                                X[:, c0:c0 + cw],
                                lhsT=WeC_sb[:, l * 128:(l + 1) * 128],
                                rhs=ea_t[:, c0:c0 + cw], start=True, stop=False)
                        for c0 in range(0, gne, 512):
                            cw = min(512, gne - c0)
                            nc.tensor.matmul(
                                X[0:64, c0:c0 + cw],
                                lhsT=ksb[:, t * 64:(t + 1) * 64],
                                rhs=R_sb[w][:, c0:c0 + cw], start=False, stop=False)
                        for c0 in range(0, gne, 512):
                            cw = min(512, gne - c0)
                            nc.tensor.matmul(
                                X[:, c0:c0 + cw], lhsT=I128_sb[:],
                                rhs=qv_f[:, jg + c0:jg + c0 + cw],
                                start=False, stop=True)
                        jg += gne
                        gate = sp.tile([64, 1024], BF16, tag="gate")
                        nc.scalar.activation(gate[:, :gne], X[0:64, :gne], AF.Silu)
                        msg = sp.tile([64, 1024], BF16, tag="msg")
                        nc.vector.tensor_tensor(out=msg[:, :gne], in0=gate[:, :gne],
                                                in1=X[64:128, :gne], op=ALU.mult)
                        mview = msg[:, :gne].rearrange("p (n w) -> p n w", w=w)
                        if gi == pl.tile_groups[t][0]:
                            nc.vector.tensor_reduce(
                                out=agg[:, t * 64:(t + 1) * 64], in_=mview,
                                axis=mybir.AxisListType.X, op=ALU.add)
                        else:
                            part = mp.tile([64, 64], F32, tag="part")
                            nc.vector.tensor_reduce(
                                out=part[:], in_=mview,
                                axis=mybir.AxisListType.X, op=ALU.add)
                            nc.vector.tensor_tensor(
                                out=agg[:, t * 64:(t + 1) * 64],
                                in0=agg[:, t * 64:(t + 1) * 64],
                                in1=part[:], op=ALU.add)

                if debug_taps and l == 0:
                    nc.sync.dma_start(out=d_agg0[:, :], in_=agg[:])

                # ---- skip connection + conv bias: agg += h @ Wskip + b
                for c0 in range(0, npad, 512):
                    cw = min(512, npad - c0)
                    ps = psB.tile([64, 512], F32, tag="nps")
                    nc.tensor.matmul(ps[:, :cw],
                                     lhsT=Wsk_sb[:, l * 64:(l + 1) * 64],
                                     rhs=h[:, c0:c0 + cw], start=True, stop=True)
                    nc.vector.tensor_tensor(out=agg[:, c0:c0 + cw],
                                            in0=agg[:, c0:c0 + cw],
                                            in1=ps[:, :cw], op=ALU.add)

                # ---- batchnorm (global over nodes)
                st = mp.tile([64, 2], F32, tag="stats")
                nc.vector.tensor_reduce(out=st[:, 0:1], in_=agg[:],
                                        axis=mybir.AxisListType.X, op=ALU.add)
                nchunk = (npad + 511) // 512
                ssbuf = mp.tile([64, 32], F32, tag="ssbuf")
                for ci in range(nchunk):
                    c0 = ci * 512
                    cw = min(512, npad - c0)
                    sqs = sp.tile([64, 512], BF16, tag="sqscratch")
                    nc.scalar.activation(sqs[:, :cw], agg[:, c0:c0 + cw],
                                         AF.Square, accum_out=ssbuf[:, ci:ci + 1])
                nc.vector.tensor_reduce(out=st[:, 1:2], in_=ssbuf[:, :nchunk],
                                        axis=mybir.AxisListType.X, op=ALU.add)
                nc.sync.dma_start(out=bn_in[:, :], in_=st[:])
                nc.gpsimd.collective_compute(
                    "AllReduce", ALU.add, replica_groups=rg,
                    ins=[bn_in.ap().opt()], outs=[bn_out.ap().opt()])
                sg = mp.tile([64, 2], F32, tag="stats2")
                nc.sync.dma_start(out=sg[:], in_=bn_out[:, :])
                mean = mp.tile([64, 1], F32, tag="c1")
                ex2 = mp.tile([64, 1], F32, tag="c2")
                var = mp.tile([64, 1], F32, tag="c3")
                rstd = mp.tile([64, 1], F32, tag="c4")
                scl = mp.tile([64, 1], F32, tag="c5")
                sht = mp.tile([64, 1], F32, tag="c6")
                inv_n = 1.0 / float(pl.n_nodes)
                nc.vector.tensor_scalar_mul(mean[:], sg[:, 0:1], inv_n)
                nc.vector.tensor_scalar_mul(ex2[:], sg[:, 1:2], inv_n)
                nc.vector.tensor_tensor(out=var[:], in0=mean[:], in1=mean[:], op=ALU.mult)
                nc.vector.tensor_tensor(out=var[:], in0=ex2[:], in1=var[:], op=ALU.subtract)
                nc.vector.tensor_scalar_add(var[:], var[:], BN_EPS)
                nc.scalar.activation(var[:], var[:], AF.Sqrt)
                nc.vector.reciprocal(rstd[:], var[:])
                nc.vector.tensor_tensor(out=scl[:], in0=rstd[:],
                                        in1=gamma_sb[:, l:l + 1], op=ALU.mult)
                nc.vector.tensor_tensor(out=sht[:], in0=mean[:], in1=scl[:], op=ALU.mult)
                nc.vector.tensor_tensor(out=sht[:], in0=beta_sb[:, l:l + 1],
                                        in1=sht[:], op=ALU.subtract)
                # h = agg * scale + shift  (cast to bf16), re-zero pad columns
                nc.scalar.activation(h[0:64, :], agg[:], AF.Identity,
                                     bias=sht[:], scale=scl[:])
                nc.vector.memset(h[0:64, pl.n_loc:npad], 0.0)
                if debug_taps and l == 0:
                    nc.sync.dma_start(out=d_h1[:, :], in_=h[:])

            # ---- head: mean pool + 2-layer MLP
            pps = psA.tile([64, NG], F32, tag="poolps", bufs=1)
            for pt in range(n_pt):
                tps = psB.tile([128, 64], F32, tag="nps")
                nc.tensor.matmul(tps[:], lhsT=h[0:64, pt * 128:(pt + 1) * 128],
                                 rhs=I64_sb[:], start=True, stop=True)
                hnm = mp.tile([128, 64], BF16, tag="hnm")
                nc.vector.tensor_copy(out=hnm[:], in_=tps[:])
                g_t = sp.tile([128, NG], BF16, tag="gtile")
                nc.sync.dma_start(out=g_t[:], in_=p_G[pt * 128:(pt + 1) * 128, :])
                nc.tensor.matmul(pps[:], lhsT=hnm[:], rhs=g_t[:],
                                 start=(pt == 0), stop=(pt == n_pt - 1))
            psum_sb = mp.tile([64, NG], F32, tag="poolsb")
            nc.vector.tensor_copy(out=psum_sb[:], in_=pps[:])
            nc.sync.dma_start(out=pool_in[:, :], in_=psum_sb[:])
            nc.gpsimd.collective_compute(
                "AllReduce", ALU.add, replica_groups=rg,
                ins=[pool_in.ap().opt()], outs=[pool_out.ap().opt()])
            pooled = mp.tile([64, NG], F32, tag="pooled")
            nc.sync.dma_start(out=pooled[:], in_=pool_out[:, :])
            recip_sb = mp.tile([64, NG], F32, tag="recipsb")
            nc.sync.dma_start(out=recip_sb[:], in_=p_recip[:, :])
            gmean = mp.tile([64, NG], BF16, tag="gmean")
            nc.vector.tensor_tensor(out=gmean[:], in0=pooled[:], in1=recip_sb[:],
                                    op=ALU.mult)
            pw_sb = mp.tile([64, 64], BF16, tag="pw")
            nc.sync.dma_start(out=pw_sb[:], in_=p_postw[:, :])
            pb_sb = mp.tile([64, 1], F32, tag="pb")
            nc.sync.dma_start(out=pb_sb[:], in_=p_postb[:, :])
            ow_sb = mp.tile([64, 1], BF16, tag="ow")
            nc.sync.dma_start(out=ow_sb[:], in_=p_outw[:, :])
            g1ps = psB.tile([64, NG], F32, tag="nps")
            nc.tensor.matmul(g1ps[:], lhsT=pw_sb[:], rhs=gmean[:], start=True, stop=True)
            g1 = mp.tile([64, NG], BF16, tag="g1")
            nc.scalar.activation(g1[:], g1ps[:], AF.Silu, bias=pb_sb[:])
            ops_ = psB.tile([1, NG], F32, tag="nps")
            nc.tensor.matmul(ops_[:], lhsT=ow_sb[:], rhs=g1[:], start=True, stop=True)
            fin = mp.tile([1, NG], F32, tag="fin")
            nc.vector.tensor_scalar_add(fin[:], ops_[:], float(out_b_val))
            nc.sync.dma_start(out=p_out[:, :], in_=fin[:])

    nc.compile()
    return nc


# ----------------------------------------------------------------------------
# entry point
# ----------------------------------------------------------------------------
def kernel(x, edge_index, edge_attr, batch_idx, pre_w, pre_b, Wk, bk, Wq, bq,
           Wv, bv, We, Wskip, conv_bias, bn_gamma, bn_beta, post_w, post_b,
           out_w, out_b, num_graphs):
    n_nodes = x.shape[0]
    n_graphs = int(num_graphs)
    pl = _build_plan(edge_index, batch_idx, n_nodes, n_graphs)
    weights = (pre_w, pre_b, Wk, bk, Wq, bq, Wv, bv, We, Wskip, conv_bias,
               bn_gamma, bn_beta, post_w, post_b, out_w, out_b)
    per_core = _host_inputs(pl, x, edge_attr, weights)
    nc = _build_kernel(pl, float(np.asarray(out_b).reshape(-1)[0]))
    res = run_bass_kernel_spmd(nc, per_core, core_ids=list(range(P_CORES)))
    out = np.asarray(res.results[0]["out"], dtype=np.float32).reshape(-1)
    return out[:n_graphs]


# revision 29
# speedup vs baseline: 1.0094x; 1.0094x over previous
"""Trainium2 Bass kernel for nn_ALIGNN (ResGatedGraphConv GNN, 3 layers).

Strategy (8 NeuronCores, SPMD):
  - Nodes are globally sorted by in-degree and dealt round-robin to cores so
    every core holds 12500 nodes whose degree profile is identical across
    cores -> one uniform program for all cores (SPMD requirement).
  - Per core, nodes form 64-wide tiles; each tile gets a "slot grid" sized by
    the tile's max degree (uniform across cores by construction).  Edges are
    laid out slot-innermost (edge position = n*w + s inside a slot-group of
    width w), so the per-node message sum is an innermost-axis tensor_reduce.
  - Per-edge compute is FEATURE-MAJOR ([feature, edge] tiles):
      PSUM X[0:64 ,:] = 2*e + k'[dst] + q'[src]     (gate argument)
      PSUM X[64:128,:] = e + v'[src]                 (value argument)
    built by three accumulating matmuls:
      MM1: lhsT=[2We|We] (50x128)      rhs = edge_attr^T chunk (moving)
      MM2: lhsT=k' node-major slice    rhs = block-diag expansion constant
      MM3: lhsT=I128                   rhs = gathered qv (feature-major)
    then gate = ACT silu(X[0:64]), msg = DVE gate * X[64:128],
    agg = DVE tensor_reduce(msg, innermost slot axis).
  - q'/v' for all nodes live in a bf16 [8*12544, 128] table ([q'|v'] rows,
    biases folded in), rebuilt each layer from local h via matmul and
    AllGather.  Per-edge rows are fetched with a two-stage dma_gather
    (int16 indices): 4 banked DRAM row-gathers land edge rows bank-sorted
    in SBUF, then one SBUF-source transpose-gather rearranges them into
    grid order, feature-major.  Bank capacities are padded to the max
    over cores so the program stays uniform (SPMD).
  - BatchNorm: local sum/sumsq -> [64,2] AllReduce -> scale/shift applied by
    one ACT op.  Mean-pool: PE transpose of h + matmul with 0/1 graph matrix,
    [64,512] AllReduce, then the 2-layer head (all cores redundantly).
"""

import os
import sys

import numpy as np

sys.path.insert(0, "/opt/trn_rl_repo")

import ml_dtypes

import concourse.bass as bass
import concourse.mybir as mybir
import concourse.tile as tile
from concourse import bacc
from concourse.bass import IndirectOffsetOnAxis
from concourse.bass_utils import run_bass_kernel_spmd

BF16 = mybir.dt.bfloat16
F32 = mybir.dt.float32
I32 = mybir.dt.int32
I16 = mybir.dt.int16
AF = mybir.ActivationFunctionType
ALU = mybir.AluOpType

P_CORES = 8
BN_EPS = 1e-5


# ----------------------------------------------------------------------------
# host-side planning
# ----------------------------------------------------------------------------
class Plan:
    pass


def _build_plan(edge_index, batch_idx, n_nodes, n_graphs):
    """Pure-numpy partitioning/geometry plan. Uniform across cores."""
    pl = Plan()
    src = np.asarray(edge_index[0], dtype=np.int64)
    dst = np.asarray(edge_index[1], dtype=np.int64)
    batch_idx = np.asarray(batch_idx, dtype=np.int64)

    deg = np.bincount(dst, minlength=n_nodes)
    order = np.argsort(deg, kind="stable")  # ascending degree
    rank_of = np.empty(n_nodes, dtype=np.int64)
    rank_of[order] = np.arange(n_nodes)

    core_of = rank_of % P_CORES
    local_of = rank_of // P_CORES  # 0..12499
    n_loc = n_nodes // P_CORES  # 12500
    n_tiles = (n_loc + 63) // 64  # 196
    npad = n_tiles * 64  # 12544
    table_row = core_of * npad + local_of

    # uniform tile degrees from the global sorted sequence
    d_t = np.zeros(n_tiles, dtype=np.int64)
    for t in range(n_tiles):
        win = order[t * 64 * P_CORES: (t + 1) * 64 * P_CORES]
        d = int(deg[win].max()) if len(win) else 2
        d = max(d, 2)
        d_t[t] = d + (d % 2)  # round up to even

    # slot-groups per tile: widths w (even, <=16)
    groups = []  # (tile, w, slot_base, edge_base)
    ebase = 0
    tile_groups = [[] for _ in range(n_tiles)]
    for t in range(n_tiles):
        rem, sbase = int(d_t[t]), 0
        while rem > 0:
            w = min(rem, 16)
            gi = len(groups)
            groups.append((t, w, sbase, ebase))
            tile_groups[t].append(gi)
            ebase += 64 * w
            sbase += w
            rem -= w
    e_pad = ebase

    # supergroups: consecutive groups packed to <=4096 edges
    sgs = []  # (edge_base, n_edges, [group indices])
    cur, cur_base, cur_len = [], 0, 0
    for gi, (t, w, sb, eb) in enumerate(groups):
        ne = 64 * w
        if cur and cur_len + ne > 4096:
            sgs.append((cur_base, cur_len, cur))
            cur, cur_base, cur_len = [], eb, 0
        if not cur:
            cur_base = eb
        cur.append(gi)
        cur_len += ne
    if cur:
        sgs.append((cur_base, cur_len, cur))

    pl.n_nodes, pl.n_graphs = n_nodes, n_graphs
    pl.n_loc, pl.n_tiles, pl.npad, pl.e_pad = n_loc, n_tiles, npad, e_pad
    pl.d_t, pl.groups, pl.tile_groups, pl.sgs = d_t, groups, tile_groups, sgs
    pl.zero_row = n_loc  # core-0 pad row: always-zero table row
    pl.order, pl.core_of, pl.local_of, pl.table_row = order, core_of, local_of, table_row
    pl.src, pl.dst, pl.batch = src, dst, batch_idx
    pl.ws = sorted(set(w for (_, w, _, _) in groups))

    # per-core edge placement (needed for uniform gather-bank capacities)
    pl.placed = [_place_edges_for_core(pl, c) for c in range(P_CORES)]
    BANK = 32768
    table_rows = P_CORES * npad
    pl.n_banks = (table_rows + BANK - 1) // BANK
    # an always-zero table row inside each bank (some core's pad region)
    pl.bank_zero = []
    for b in range(pl.n_banks):
        zr = None
        for c in range(P_CORES):
            for r in range(c * npad + n_loc, (c + 1) * npad):
                if b * BANK <= r < (b + 1) * BANK:
                    zr = r - b * BANK
                    break
            if zr is not None:
                break
        assert zr is not None, f"no zero row in bank {b}"
        pl.bank_zero.append(zr)
    # per (sg, bank): uniform dest capacity (128-blocks) = max over cores
    pl.sg_bank = []  # list per sg: (caps[b] in blocks, offsets[b] in blocks)
    for (eb, ne, gis) in sgs:
        caps = []
        for b in range(pl.n_banks):
            mx = 0
            for c in range(P_CORES):
                srow = pl.placed[c][1][eb:eb + ne]
                mx = max(mx, int((srow // BANK == b).sum()))
            caps.append((mx + 127) // 128)
        offs = np.concatenate([[0], np.cumsum(caps)])[:-1].tolist()
        pl.sg_bank.append((caps, offs))
    return pl


def _place_edges_for_core(pl, c):
    """Return (ea_eid[e_pad] int64 (-1 dummy), src_row[e_pad] int32)."""
    sel = np.nonzero(pl.core_of[pl.dst] == c)[0]
    li = pl.local_of[pl.dst[sel]]
    o = np.argsort(li, kind="stable")
    sel, li = sel[o], li[o]
    # position within each node's edge list
    first = np.searchsorted(li, np.arange(pl.n_loc))
    pos = np.arange(len(li)) - first[li]

    t = li // 64
    n = li % 64
    ea_eid = np.full(pl.e_pad, -1, dtype=np.int64)
    src_row = np.full(pl.e_pad, pl.zero_row, dtype=np.int32)

    # map slot j -> (group, s) per tile via per-tile group tables
    flat = np.empty(len(li), dtype=np.int64)
    for ti in range(pl.n_tiles):
        m = np.nonzero(t == ti)[0]
        if len(m) == 0:
            continue
        jj = pos[m]
        gidx = np.zeros(len(m), dtype=np.int64)
        gs = np.zeros(len(m), dtype=np.int64)
        for gi in pl.tile_groups[ti]:
            (_, w, sb, eb) = pl.groups[gi]
            inw = (jj >= sb) & (jj < sb + w)
            gidx[inw] = gi
            gs[inw] = jj[inw] - sb
        ebs = np.array([pl.groups[g][3] for g in gidx])
        wg = np.array([pl.groups[g][1] for g in gidx])
        flat[m] = ebs + n[m] * wg + gs
    ea_eid[flat] = sel
    src_row[flat] = pl.table_row[pl.src[sel]].astype(np.int32)
    return ea_eid, src_row


def _bf16(a):
    return np.asarray(a, dtype=np.float32).astype(ml_dtypes.bfloat16)


def _wrap16(vals):
    """int16 list (len % 16 == 0) -> [128, n/16] wrapped+replicated layout."""
    n = len(vals)
    assert n % 16 == 0
    w = np.asarray(vals, np.int16).reshape(n // 16, 16).T
    return np.ascontiguousarray(np.tile(w, (8, 1)))


def _host_inputs(pl, x, edge_attr, weights):
    """Build per-core input dicts (shared entries replicated)."""
    (pre_w, pre_b, Wk, bk, Wq, bq, Wv, bv, We, Wskip, conv_bias,
     bn_gamma, bn_beta, post_w, post_b, out_w, out_b) = weights
    L = Wk.shape[0]
    npad, n_loc = pl.npad, pl.n_loc

    shared = {}
    # [2We | We] per layer, stacked on columns: [50, L*128]
    WeC = np.concatenate(
        [np.concatenate([2.0 * We[l], We[l]], axis=1) for l in range(L)], axis=1)
    shared["WeC"] = _bf16(WeC)
    shared["Wk_all"] = _bf16(np.concatenate([Wk[l] for l in range(L)], axis=1))
    # Wqv aug: rows 0..63 = [Wq|Wv], row 64 = [bq+bk | bv]
    Wqv = np.concatenate(
        [np.concatenate(
            [np.concatenate([Wq[l], Wv[l]], axis=1),
             np.concatenate([bq[l] + bk[l], bv[l]])[None, :]], axis=0)
         for l in range(L)], axis=1)
    shared["Wqv_all"] = _bf16(Wqv)  # [65, L*128]
    Wsk = np.concatenate(
        [np.concatenate([Wskip[l], conv_bias[l][None, :]], axis=0)
         for l in range(L)], axis=1)
    shared["Wskip_all"] = _bf16(Wsk)  # [65, L*64]
    shared["preW"] = _bf16(np.concatenate([pre_w, pre_b[None, :]], axis=0))  # [93,64]
    shared["gamma"] = np.ascontiguousarray(bn_gamma.T).astype(np.float32)  # [64, L]
    shared["beta"] = np.ascontiguousarray(bn_beta.T).astype(np.float32)
    shared["post_w"] = _bf16(post_w)  # [64,64]
    shared["post_b"] = np.asarray(post_b, np.float32).reshape(64, 1)
    shared["out_w"] = _bf16(out_w)  # [64,1]

    cnts = np.bincount(pl.batch, minlength=pl.n_graphs).astype(np.float32)
    recip = 1.0 / np.maximum(cnts, 1.0)
    shared["recip"] = np.broadcast_to(recip[None, :], (64, pl.n_graphs)).astype(np.float32).copy()

    eaT_f = np.asarray(edge_attr, np.float32)
    xT_f = np.asarray(x, np.float32)

    per_core = []
    for c in range(P_CORES):
        d = dict(shared)
        ea_eid, src_row = pl.placed[c]
        eaT = np.zeros((50, pl.e_pad), dtype=ml_dtypes.bfloat16)
        m = ea_eid >= 0
        eaT[:, m] = _bf16(eaT_f[ea_eid[m]].T)
        d["eaT"] = eaT
        # two-stage gather indices
        BANK = 32768
        g1_cols, g2_cols = [], []
        for si, (eb, ne, gis) in enumerate(pl.sgs):
            caps, offs = pl.sg_bank[si]
            srow = src_row[eb:eb + ne]
            bank = srow // BANK
            pos_in_stage1 = np.zeros(ne, dtype=np.int16)
            for b in range(pl.n_banks):
                if caps[b] == 0:
                    continue
                cap = caps[b] * 128
                m = np.nonzero(bank == b)[0]
                v = np.full(cap, pl.bank_zero[b], dtype=np.int16)
                v[:len(m)] = (srow[m] - b * BANK).astype(np.int16)
                g1_cols.append(_wrap16(v))
                pos_in_stage1[m] = offs[b] * 128 + np.arange(len(m), dtype=np.int16)
            g2_cols.append(_wrap16(pos_in_stage1))
        d["g1idx"] = np.concatenate(g1_cols, axis=1)
        d["g2idx"] = np.concatenate(g2_cols, axis=1)
        # node features, transposed + ones row; pad cols zero
        xT = np.zeros((93, npad), dtype=ml_dtypes.bfloat16)
        glb = pl.order[np.arange(n_loc) * P_CORES + c]
        xT[:92, :n_loc] = _bf16(xT_f[glb].T)
        xT[92, :n_loc] = ml_dtypes.bfloat16(1.0)
        d["xT"] = xT
        # pool matrix [npad, n_graphs]
        G = np.zeros((npad, pl.n_graphs), dtype=ml_dtypes.bfloat16)
        G[np.arange(n_loc), pl.batch[glb]] = ml_dtypes.bfloat16(1.0)
        d["G"] = G
        per_core.append(d)
    return per_core


# ----------------------------------------------------------------------------
# device kernel builder
# ----------------------------------------------------------------------------
def _build_kernel(pl, out_b_val, debug_taps=False):
    L = 3
    npad, n_tiles, e_pad = pl.npad, pl.n_tiles, pl.e_pad
    n_pt = n_tiles // 2  # 128-node production tiles
    NG = pl.n_graphs
    max_cap_blocks = max(sum(caps) for (caps, offs) in pl.sg_bank)

    nc = bacc.Bacc(None, target_bir_lowering=False, num_devices=P_CORES)
    rg = [list(range(P_CORES))]

    # ---- parameters
    p_WeC = nc.declare_dram_parameter("WeC", [50, L * 128], BF16, isOutput=False)
    p_Wk = nc.declare_dram_parameter("Wk_all", [64, L * 64], BF16, isOutput=False)
    p_Wqv = nc.declare_dram_parameter("Wqv_all", [65, L * 128], BF16, isOutput=False)
    p_Wsk = nc.declare_dram_parameter("Wskip_all", [65, L * 64], BF16, isOutput=False)
    p_preW = nc.declare_dram_parameter("preW", [93, 64], BF16, isOutput=False)
    p_gamma = nc.declare_dram_parameter("gamma", [64, L], F32, isOutput=False)
    p_beta = nc.declare_dram_parameter("beta", [64, L], F32, isOutput=False)
    p_postw = nc.declare_dram_parameter("post_w", [64, 64], BF16, isOutput=False)
    p_postb = nc.declare_dram_parameter("post_b", [64, 1], F32, isOutput=False)
    p_outw = nc.declare_dram_parameter("out_w", [64, 1], BF16, isOutput=False)
    p_recip = nc.declare_dram_parameter("recip", [64, NG], F32, isOutput=False)
    p_eaT = nc.declare_dram_parameter("eaT", [50, e_pad], BF16, isOutput=False)
    g1_cols_total = sum(sum(caps) * 8 for (caps, offs) in pl.sg_bank)
    g2_cols_total = sum(ne // 16 for (_, ne, _) in pl.sgs)
    p_g1 = nc.declare_dram_parameter("g1idx", [128, g1_cols_total], I16, isOutput=False)
    p_g2 = nc.declare_dram_parameter("g2idx", [128, g2_cols_total], I16, isOutput=False)
    p_xT = nc.declare_dram_parameter("xT", [93, npad], BF16, isOutput=False)
    p_G = nc.declare_dram_parameter("G", [npad, NG], BF16, isOutput=False)
    p_out = nc.declare_dram_parameter("out", [1, NG], F32, isOutput=True)
    if debug_taps:
        d_qv0 = nc.declare_dram_parameter("dbg_qv0", [128, 4096], BF16, isOutput=True)
        d_agg0 = nc.declare_dram_parameter("dbg_agg0", [64, npad], F32, isOutput=True)
        d_h1 = nc.declare_dram_parameter("dbg_h1", [65, npad], BF16, isOutput=True)
        d_ksb0 = nc.declare_dram_parameter("dbg_ksb0", [64, n_tiles * 64], BF16, isOutput=True)
        d_tab0 = nc.declare_dram_parameter("dbg_tab0", [P_CORES * npad, 128], BF16, isOutput=True)

    # ---- internal DRAM
    qv_shard = nc.dram_tensor("qv_shard", [npad, 128], BF16)
    qv_table = nc.dram_tensor("qv_table", [P_CORES * npad, 128], BF16, addr_space="Shared")
    bn_in = nc.dram_tensor("bn_in", [64, 2], F32)
    bn_out = nc.dram_tensor("bn_out", [64, 2], F32, addr_space="Shared")
    pool_in = nc.dram_tensor("pool_in", [64, NG], F32)
    pool_out = nc.dram_tensor("pool_out", [64, NG], F32, addr_space="Shared")

    # inline constants
    ident128 = nc.inline_tensor(np.eye(128, dtype=ml_dtypes.bfloat16), name="I128")
    ident64 = nc.inline_tensor(np.eye(64, dtype=ml_dtypes.bfloat16), name="I64")
    r_pats = {}
    for w in pl.ws:
        R = np.zeros((64, 64 * w), dtype=ml_dtypes.bfloat16)
        for n in range(64):
            R[n, n * w:(n + 1) * w] = ml_dtypes.bfloat16(1.0)
        r_pats[w] = nc.inline_tensor(R, name=f"Rpat{w}")

    with tile.TileContext(nc) as tc:
        with (
            tc.tile_pool(name="persist", bufs=1) as pp,
            tc.tile_pool(name="stream", bufs=3) as sp,
            tc.tile_pool(name="gath", bufs=2) as gp,
            tc.tile_pool(name="small", bufs=2) as mp,
            tc.tile_pool(name="psA", bufs=2, space="PSUM") as psA,
            tc.tile_pool(name="psB", bufs=2, space="PSUM") as psB,
        ):
            # ---- persistent SBUF
            h = pp.tile([65, npad], BF16, tag="h")
            ksb = pp.tile([64, n_tiles * 64], BF16, tag="ksb")
            agg = pp.tile([64, npad], F32, tag="agg")
            WeC_sb = pp.tile([50, L * 128], BF16, tag="WeC")
            Wk_sb = pp.tile([64, L * 64], BF16, tag="Wk")
            Wqv_sb = pp.tile([65, L * 128], BF16, tag="Wqv")
            Wsk_sb = pp.tile([65, L * 64], BF16, tag="Wsk")
            preW_sb = pp.tile([93, 64], BF16, tag="preW")
            I128_sb = pp.tile([128, 128], BF16, tag="I128")
            I64_sb = pp.tile([64, 64], BF16, tag="I64")
            R_sb = {w: pp.tile([64, 64 * w], BF16, tag=f"R{w}", name=f"R{w}")
                    for w in pl.ws}
            gamma_sb = pp.tile([64, L], F32, tag="gamma")
            beta_sb = pp.tile([64, L], F32, tag="beta")

            nc.sync.dma_start(out=WeC_sb[:], in_=p_WeC[:, :])
            nc.sync.dma_start(out=Wk_sb[:], in_=p_Wk[:, :])
            nc.sync.dma_start(out=Wqv_sb[:], in_=p_Wqv[:, :])
            nc.sync.dma_start(out=Wsk_sb[:], in_=p_Wsk[:, :])
            nc.sync.dma_start(out=preW_sb[:], in_=p_preW[:, :])
            nc.sync.dma_start(out=I128_sb[:], in_=ident128[:, :])
            nc.sync.dma_start(out=I64_sb[:], in_=ident64[:, :])
            for w in pl.ws:
                nc.sync.dma_start(out=R_sb[w][:], in_=r_pats[w][:, :])
            nc.sync.dma_start(out=gamma_sb[:], in_=p_gamma[:, :])
            nc.sync.dma_start(out=beta_sb[:], in_=p_beta[:, :])

            # ---- pre-FC: h = silu(x @ pre_w + pre_b), xT streamed per chunk
            for c0 in range(0, npad, 512):
                cw = min(512, npad - c0)
                xt = sp.tile([93, 512], BF16, tag="xtile")
                nc.sync.dma_start(out=xt[:, :cw], in_=p_xT[:, c0:c0 + cw])
                ps = psB.tile([64, 512], F32, tag="nps")
                nc.tensor.matmul(ps[:, :cw], lhsT=preW_sb[:],
                                 rhs=xt[:, :cw], start=True, stop=True)
                nc.scalar.activation(h[0:64, c0:c0 + cw], ps[:, :cw], AF.Silu)
            nc.vector.memset(h[64:65, 0:pl.n_loc], 1.0)
            nc.vector.memset(h[64:65, pl.n_loc:npad], 0.0)
            nc.vector.memset(h[0:64, pl.n_loc:npad], 0.0)

            for l in range(L):
                # ---- node phase: qv table shard + k'
                for pt in range(n_pt):
                    cols = slice(pt * 128, pt * 128 + 128)
                    qps = psB.tile([128, 128], F32, tag="nps")
                    nc.tensor.matmul(qps[:], lhsT=h[:, cols],
                                     rhs=Wqv_sb[:, l * 128:(l + 1) * 128],
                                     start=True, stop=True)
                    qsb = mp.tile([128, 128], BF16, tag="qvsb")
                    nc.vector.tensor_copy(out=qsb[:], in_=qps[:])
                    nc.sync.dma_start(out=qv_shard[pt * 128:(pt + 1) * 128, :], in_=qsb[:])
                for t in range(n_tiles):
                    kps = psB.tile([64, 64], F32, tag="nps")
                    nc.tensor.matmul(kps[:], lhsT=h[0:64, t * 64:(t + 1) * 64],
                                     rhs=Wk_sb[:, l * 64:(l + 1) * 64],
                                     start=True, stop=True)
                    nc.vector.tensor_copy(out=ksb[:, t * 64:(t + 1) * 64], in_=kps[:])

                nc.gpsimd.collective_compute(
                    "AllGather", ALU.bypass, replica_groups=rg,
                    ins=[qv_shard.ap().opt()], outs=[qv_table.ap().opt()])
                if debug_taps and l == 0:
                    nc.sync.dma_start(out=d_tab0[:, :], in_=qv_table[:, :])
                    nc.sync.dma_start(out=d_ksb0[:, :], in_=ksb[:])

                # ---- edge phase
                k1off = 0
                k2off = 0
                for sgi, (eb, ne, gis) in enumerate(pl.sgs):
                    caps, offs = pl.sg_bank[sgi]
                    tot_blocks = sum(caps)
                    # stage 1: banked DRAM row-gathers (int16) -> edge-major
                    qv_t = gp.tile([128, max_cap_blocks, 128], BF16, tag="qvt")
                    for b in range(pl.n_banks):
                        if caps[b] == 0:
                            continue
                        nidx = caps[b] * 128
                        i1 = sp.tile([128, 512], I16, tag="idx1")
                        nc.sync.dma_start(out=i1[:, :nidx // 16],
                                          in_=p_g1[:, k1off:k1off + nidx // 16])
                        nc.gpsimd.dma_gather(
                            out_ap=qv_t[:, offs[b]:offs[b] + caps[b], :],
                            in_ap=qv_table[b * 32768:
                                           min((b + 1) * 32768, P_CORES * npad), :],
                            idxs_ap=i1[:, :nidx // 16],
                            num_idxs=nidx, num_idxs_reg=nidx, elem_size=128,
                            single_packet=False)
                        k1off += nidx // 16
                    # stage 2: SBUF re-gather into grid order, feature-major
                    i2 = sp.tile([128, 256], I16, tag="idx2")
                    nc.sync.dma_start(out=i2[:, :ne // 16],
                                      in_=p_g2[:, k2off:k2off + ne // 16])
                    qv_f = gp.tile([128, 4096], BF16, tag="qvf")
                    nc.gpsimd.dma_gather(
                        out_ap=qv_f[:, :ne].rearrange("p (o n) -> p o n", o=1),
                        in_ap=qv_t[:, :tot_blocks, :],
                        idxs_ap=i2[:, :ne // 16],
                        num_idxs=ne, num_idxs_reg=ne, elem_size=128,
                        transpose=True, sbuf_tokens_per_rank=128,
                        sbuf_free_dim_per_rank=256, single_packet=False)
                    k2off += ne // 16
                    if debug_taps and l == 0 and sgi == 0:
                        nc.sync.dma_start(out=d_qv0[:, :ne], in_=qv_f[:, :ne])
                    jg = 0  # 128-col block offset within supergroup
                    for gi in gis:
                        (t, w, sb, geb) = pl.groups[gi]
                        gne = 64 * w
                        X = psA.tile([128, 1024], F32, tag="X")
                        ea_t = sp.tile([50, 1024], BF16, tag="ea")
                        nc.sync.dma_start(out=ea_t[:, :gne], in_=p_eaT[:, geb:geb + gne])
                        for c0 in range(0, gne, 512):
                            cw = min(512, gne - c0)
                            nc.tensor.matmul(
                                X[:, c0:c0 + cw],
                                lhsT=WeC_sb[:, l * 128:(l + 1) * 128],
                                rhs=ea_t[:, c0:c0 + cw], start=True, stop=False)
                        for c0 in range(0, gne, 512):
                            cw = min(512, gne - c0)
                            nc.tensor.matmul(
                                X[0:64, c0:c0 + cw],
                                lhsT=ksb[:, t * 64:(t + 1) * 64],
                                rhs=R_sb[w][:, c0:c0 + cw], start=False, stop=False)
                        for c0 in range(0, gne, 512):
                            cw = min(512, gne - c0)
                            nc.tensor.matmul(
                                X[:, c0:c0 + cw], lhsT=I128_sb[:],
                                rhs=qv_f[:, jg + c0:jg + c0 + cw],
                                start=False, stop=True)
                        jg += gne
                        gate = sp.tile([64, 1024], BF16, tag="gate")
                        nc.scalar.activation(gate[:, :gne], X[0:64, :gne], AF.Silu)
                        msg = sp.tile([64, 1024], BF16, tag="msg")
                        nc.vector.tensor_tensor(out=msg[:, :gne], in0=gate[:, :gne],
                                                in1=X[64:128, :gne], op=ALU.mult)
                        mview = msg[:, :gne].rearrange("p (n w) -> p n w", w=w)
                        if gi == pl.tile_groups[t][0]:
                            nc.vector.tensor_reduce(
                                out=agg[:, t * 64:(t + 1) * 64], in_=mview,
                                axis=mybir.AxisListType.X, op=ALU.add)
                        else:
                            part = mp.tile([64, 64], F32, tag="part")
                            nc.vector.tensor_reduce(
                                out=part[:], in_=mview,
                                axis=mybir.AxisListType.X, op=ALU.add)
                            nc.vector.tensor_tensor(
                                out=agg[:, t * 64:(t + 1) * 64],
                                in0=agg[:, t * 64:(t + 1) * 64],
                                in1=part[:], op=ALU.add)

                if debug_taps and l == 0:
                    nc.sync.dma_start(out=d_agg0[:, :], in_=agg[:])

                # ---- skip connection + conv bias: agg += h @ Wskip + b
                for c0 in range(0, npad, 512):
                    cw = min(512, npad - c0)
                    ps = psB.tile([64, 512], F32, tag="nps")
                    nc.tensor.matmul(ps[:, :cw],
                                     lhsT=Wsk_sb[:, l * 64:(l + 1) * 64],
                                     rhs=h[:, c0:c0 + cw], start=True, stop=True)
                    nc.vector.tensor_tensor(out=agg[:, c0:c0 + cw],
                                            in0=agg[:, c0:c0 + cw],
                                            in1=ps[:, :cw], op=ALU.add)

                # ---- batchnorm (global over nodes)
                st = mp.tile([64, 2], F32, tag="stats")
                nc.vector.tensor_reduce(out=st[:, 0:1], in_=agg[:],
                                        axis=mybir.AxisListType.X, op=ALU.add)
                nchunk = (npad + 511) // 512
                ssbuf = mp.tile([64, 32], F32, tag="ssbuf")
                for ci in range(nchunk):
                    c0 = ci * 512
                    cw = min(512, npad - c0)
                    sqs = sp.tile([64, 512], BF16, tag="sqscratch")
                    nc.scalar.activation(sqs[:, :cw], agg[:, c0:c0 + cw],
                                         AF.Square, accum_out=ssbuf[:, ci:ci + 1])
                nc.vector.tensor_reduce(out=st[:, 1:2], in_=ssbuf[:, :nchunk],
                                        axis=mybir.AxisListType.X, op=ALU.add)
                nc.sync.dma_start(out=bn_in[:, :], in_=st[:])
                nc.gpsimd.collective_compute(
                    "AllReduce", ALU.add, replica_groups=rg,
                    ins=[bn_in.ap().opt()], outs=[bn_out.ap().opt()])
                sg = mp.tile([64, 2], F32, tag="stats2")
                nc.sync.dma_start(out=sg[:], in_=bn_out[:, :])
                mean = mp.tile([64, 1], F32, tag="c1")
                ex2 = mp.tile([64, 1], F32, tag="c2")
                var = mp.tile([64, 1], F32, tag="c3")
                rstd = mp.tile([64, 1], F32, tag="c4")
                scl = mp.tile([64, 1], F32, tag="c5")
                sht = mp.tile([64, 1], F32, tag="c6")
                inv_n = 1.0 / float(pl.n_nodes)
                nc.vector.tensor_scalar_mul(mean[:], sg[:, 0:1], inv_n)
                nc.vector.tensor_scalar_mul(ex2[:], sg[:, 1:2], inv_n)
                nc.vector.tensor_tensor(out=var[:], in0=mean[:], in1=mean[:], op=ALU.mult)
                nc.vector.tensor_tensor(out=var[:], in0=ex2[:], in1=var[:], op=ALU.subtract)
                nc.vector.tensor_scalar_add(var[:], var[:], BN_EPS)
                nc.scalar.activation(var[:], var[:], AF.Sqrt)
                nc.vector.reciprocal(rstd[:], var[:])
                nc.vector.tensor_tensor(out=scl[:], in0=rstd[:],
                                        in1=gamma_sb[:, l:l + 1], op=ALU.mult)
                nc.vector.tensor_tensor(out=sht[:], in0=mean[:], in1=scl[:], op=ALU.mult)
                nc.vector.tensor_tensor(out=sht[:], in0=beta_sb[:, l:l + 1],
                                        in1=sht[:], op=ALU.subtract)
                # h = agg * scale + shift  (cast to bf16), re-zero pad columns
                nc.scalar.activation(h[0:64, :], agg[:], AF.Identity,
                                     bias=sht[:], scale=scl[:])
                nc.vector.memset(h[0:64, pl.n_loc:npad], 0.0)
                if debug_taps and l == 0:
                    nc.sync.dma_start(out=d_h1[:, :], in_=h[:])

            # ---- head: mean pool + 2-layer MLP
            pps = psA.tile([64, NG], F32, tag="poolps", bufs=1)
            for pt in range(n_pt):
                tps = psB.tile([128, 64], F32, tag="nps")
                nc.tensor.matmul(tps[:], lhsT=h[0:64, pt * 128:(pt + 1) * 128],
                                 rhs=I64_sb[:], start=True, stop=True)
                hnm = mp.tile([128, 64], BF16, tag="hnm")
                nc.vector.tensor_copy(out=hnm[:], in_=tps[:])
                g_t = sp.tile([128, NG], BF16, tag="gtile")
                nc.sync.dma_start(out=g_t[:], in_=p_G[pt * 128:(pt + 1) * 128, :])
                nc.tensor.matmul(pps[:], lhsT=hnm[:], rhs=g_t[:],
                                 start=(pt == 0), stop=(pt == n_pt - 1))
            psum_sb = mp.tile([64, NG], F32, tag="poolsb")
            nc.vector.tensor_copy(out=psum_sb[:], in_=pps[:])
            nc.sync.dma_start(out=pool_in[:, :], in_=psum_sb[:])
            nc.gpsimd.collective_compute(
                "AllReduce", ALU.add, replica_groups=rg,
                ins=[pool_in.ap().opt()], outs=[pool_out.ap().opt()])
            pooled = mp.tile([64, NG], F32, tag="pooled")
            nc.sync.dma_start(out=pooled[:], in_=pool_out[:, :])
            recip_sb = mp.tile([64, NG], F32, tag="recipsb")
            nc.sync.dma_start(out=recip_sb[:], in_=p_recip[:, :])
            gmean = mp.tile([64, NG], BF16, tag="gmean")
            nc.vector.tensor_tensor(out=gmean[:], in0=pooled[:], in1=recip_sb[:],
                                    op=ALU.mult)
            pw_sb = mp.tile([64, 64], BF16, tag="pw")
            nc.sync.dma_start(out=pw_sb[:], in_=p_postw[:, :])
            pb_sb = mp.tile([64, 1], F32, tag="pb")
            nc.sync.dma_start(out=pb_sb[:], in_=p_postb[:, :])
            ow_sb = mp.tile([64, 1], BF16, tag="ow")
            nc.sync.dma_start(out=ow_sb[:], in_=p_outw[:, :])
            g1ps = psB.tile([64, NG], F32, tag="nps")
            nc.tensor.matmul(g1ps[:], lhsT=pw_sb[:], rhs=gmean[:], start=True, stop=True)
            g1 = mp.tile([64, NG], BF16, tag="g1")
            nc.scalar.activation(g1[:], g1ps[:], AF.Silu, bias=pb_sb[:])
            ops_ = psB.tile([1, NG], F32, tag="nps")
            nc.tensor.matmul(ops_[:], lhsT=ow_sb[:], rhs=g1[:], start=True, stop=True)
            fin = mp.tile([1, NG], F32, tag="fin")
            nc.vector.tensor_scalar_add(fin[:], ops_[:], float(out_b_val))
            nc.sync.dma_start(out=p_out[:, :], in_=fin[:])

    nc.compile()
    return nc


# ----------------------------------------------------------------------------
# entry point
# ----------------------------------------------------------------------------
def kernel(x, edge_index, edge_attr, batch_idx, pre_w, pre_b, Wk, bk, Wq, bq,
           Wv, bv, We, Wskip, conv_bias, bn_gamma, bn_beta, post_w, post_b,
           out_w, out_b, num_graphs):
    n_nodes = x.shape[0]
    n_graphs = int(num_graphs)
    pl = _build_plan(edge_index, batch_idx, n_nodes, n_graphs)
    weights = (pre_w, pre_b, Wk, bk, Wq, bq, Wv, bv, We, Wskip, conv_bias,
               bn_gamma, bn_beta, post_w, post_b, out_w, out_b)
    per_core = _host_inputs(pl, x, edge_attr, weights)
    nc = _build_kernel(pl, float(np.asarray(out_b).reshape(-1)[0]))
    res = run_bass_kernel_spmd(nc, per_core, core_ids=list(range(P_CORES)))
    out = np.asarray(res.results[0]["out"], dtype=np.float32).reshape(-1)
    return out[:n_graphs]
